# revision 1
# baseline (speedup 1.0000x reference)
"""Trainium2 kernel for nn_CascadedABCDCircuit: cascaded 2-port ABCD ladder.

Math: each stage multiplies the ABCD state by (I + s_i*G_i) where G_i is a
constant nilpotent 2x2 complex matrix and s_i = (omega*v_i)^{+-1} (the
complex reciprocals 1/(w*(1/Q + j)) are just const/w). So every output
component is a Laurent polynomial in omega, degree -6..+6, with
batch-dependent coefficients. Host computes the 13 coefficients per
(component, batch) exactly in fp64 via the recurrence applied to polynomial
coefficient vectors (tiny (1024,13) complex ops). The device evaluates
out[c,b,f] = sum_m C[c,b,m] * W[m,f] as K=13 matmuls and streams the
result to HBM — memory-bound, as this problem's regime demands.

Precision/bandwidth: the correctness gate is rel_err < 2e-2, so the output
is stored as bf16 (rel err ~1e-3) and upconverted to f32 on the host —
this halves the dominant HBM store traffic. Matmul operands are bf16 too:
both sides are split hi (bf16 round) + lo (residual in bf16) and the three
significant cross terms evaluate in a single K=39 matmul by stacking the
splits along the contraction dim: lhsT = [h;h;lo], rhs = [W1;W2;W1]
(~17-bit effective mantissa, far above what the bf16 store keeps). Matmul
throughput is per moving column (K only fills the array), so the
corrections are free.

Scheduling: the PE clock is pinned at 1.2 GHz on this part (verified: 55us
of gapless matmuls never unthrottle), so the kernel is bound by the PE
drain rate (65536 moving cols = 54.7us) and the goal is a perfectly dense
matmul stream with minimal head/tail. Inputs land in 7 DMAs across both
HWDGE rings ordered by first use (first matmul's operands ~300KB in, first
MM at ~10us). PSUM is cycled as 4 tiles x 2 banks with 1024-column
PSUM->SBUF copies (f32->bf16 cast) alternating DVE/ACT, which outpaces the
matmul stream so the PE never stalls on drain. Output goes out per
component (2MB DMAs, 16KB/partition descriptors) on the sync ring, with
the first component split in halves (early pipe start) and the last
tapered down to a 256KB final chunk (short tail drain).

Sharding: pure data-parallel over batch: 8 cores x 128 batches, every core
sees all 8192 freqs.
"""
import numpy as np
import sys

for _p in ("/opt/trn_rl_repo", "/root/.axon_site/_ro/trn_rl_repo"):
    if _p not in sys.path:
        sys.path.append(_p)

import ml_dtypes
import concourse.bacc as bacc
import concourse.mybir as mybir
from concourse import tile
from concourse.bass_utils import run_bass_kernel_spmd

# Problem constants (hardcoded per contract)
B, F = 1024, 8192
OP_CODES = [3, 0, 1, 2, 3, 0, 1, 2, 3, 0, 1, 2]
Q_L, Q_C = 50.0, 100.0
NK, K0 = 13, 6               # omega powers -6..+6
NCORES = 8
BPC = B // NCORES            # 128 batches per core
NCOMP = 8                    # Ar, Ai, Br, Bi, Cr, Ci, Dr, Di
OM0 = 2.0 * np.pi * np.sqrt(1e9 * 10e9)   # omega normalizer (geometric mid)

PS_CHUNK = 1024              # psum tile free dim (2 banks)
MM_N = 512                   # moving free dim per matmul (1 PSUM bank, fp32)
KS = 3 * NK                  # stacked contraction dim (39)
CB = NCOMP * BPC             # coefficient columns (1024)
BF16 = ml_dtypes.bfloat16

LAST_RESULTS = None          # BassKernelResults of the most recent run
_COMPILED = {}


def _host_coeffs(values):
    """values (B,12) fp32 -> (NCOMP, B, NK) fp64 coeffs in powers of (om/OM0)."""
    v = values.astype(np.float64)
    nb = v.shape[0]
    A = np.zeros((nb, NK), np.complex128); A[:, K0] = 1.0
    Bm = np.zeros((nb, NK), np.complex128)
    Cm = np.zeros((nb, NK), np.complex128)
    Dm = np.zeros((nb, NK), np.complex128); Dm[:, K0] = 1.0

    def shift_mul(P, fac, dk):
        out = np.zeros_like(P)
        if dk == 1:
            out[:, 1:] = P[:, :-1]
        else:
            out[:, :-1] = P[:, 1:]
        return out * fac[:, None]

    for i, code in enumerate(OP_CODES):
        vi = v[:, i]
        if code == 0:      # series L
            fac = vi * OM0 * (1.0 / Q_L + 1j)
            Bm = Bm + shift_mul(A, fac, +1)
            Dm = Dm + shift_mul(Cm, fac, +1)
        elif code == 1:    # series C (reciprocal)
            c = (1.0 / Q_C - 1j) / (1.0 + 1.0 / Q_C**2)
            fac = c / (vi * OM0)
            Bm = Bm + shift_mul(A, fac, -1)
            Dm = Dm + shift_mul(Cm, fac, -1)
        elif code == 2:    # shunt L (reciprocal)
            c = (1.0 / Q_L - 1j) / (1.0 + 1.0 / Q_L**2)
            fac = c / (vi * OM0)
            A = A + shift_mul(Bm, fac, -1)
            Cm = Cm + shift_mul(Dm, fac, -1)
        else:              # shunt C
            fac = vi * OM0 * (1.0 / Q_C + 1j)
            A = A + shift_mul(Bm, fac, +1)
            Cm = Cm + shift_mul(Dm, fac, +1)
    return np.stack([A.real, A.imag, Bm.real, Bm.imag,
                     Cm.real, Cm.imag, Dm.real, Dm.imag])


def _build_module():
    """SPMD module: cw[39, CB+F] bf16 ([Cstack|Wstack]) -> out[NCOMP, BPC, F] bf16."""
    nc = bacc.Bacc("TRN2", target_bir_lowering=False, debug=False,
                   enable_asserts=False, num_devices=NCORES)
    cw_cols = CB + F
    cw_d = nc.dram_tensor("cw", [KS, cw_cols], mybir.dt.bfloat16,
                          kind="ExternalInput")
    out_d = nc.dram_tensor("out", [NCOMP, BPC, F], mybir.dt.bfloat16,
                           kind="ExternalOutput")

    with tile.TileContext(nc) as tc:
        with (
            tc.tile_pool(name="const", bufs=1) as cpool,
            tc.tile_pool(name="stage", bufs=3) as spool,
            tc.tile_pool(name="ps", bufs=4, space="PSUM") as pspool,
        ):
            cw = cpool.tile([KS, cw_cols], mybir.dt.bfloat16)

            # Input loads across both HWDGE rings, ordered so the first
            # matmul's operands (comp-0 coefficients + first 512 W cols)
            # land first; later W chunks stream in well ahead of use.
            # (each dma_start costs ~1us of sequencer issue time, so the
            # chunking balances early start vs issue overhead)
            nc.sync.dma_start(cw[:, CB:CB + 2048], cw_d[:, CB:CB + 2048])
            nc.scalar.dma_start(cw[:, :BPC], cw_d[:, :BPC])
            nc.sync.dma_start(cw[:, CB + 2048:CB + 4096],
                              cw_d[:, CB + 2048:CB + 4096])
            nc.scalar.dma_start(cw[:, CB + 4096:CB + 6144],
                                cw_d[:, CB + 4096:CB + 6144])
            nc.sync.dma_start(cw[:, CB + 6144:], cw_d[:, CB + 6144:])
            nc.scalar.dma_start(cw[:, BPC:CB], cw_d[:, BPC:CB])

            copy_engines = [nc.vector, nc.scalar]
            ncopy = 0
            for c in range(NCOMP):
                ot = spool.tile([BPC, F], mybir.dt.bfloat16)
                lhsT = cw[:, c * BPC:(c + 1) * BPC]
                final = c == NCOMP - 1
                for ci in range(F // PS_CHUNK):
                    acc = pspool.tile([BPC, PS_CHUNK], mybir.dt.float32)
                    pos = ci * PS_CHUNK
                    for j in range(PS_CHUNK // MM_N):
                        col = CB + pos + j * MM_N
                        nc.tensor.matmul(acc[:, j * MM_N:(j + 1) * MM_N],
                                         lhsT, cw[:, col:col + MM_N])
                    # PSUM->SBUF drain with f32->bf16 cast; DVE/ACT are the
                    # only engines with a PSUM port. The very last chunk is
                    # copied as two halves on both engines in parallel so
                    # the final bytes hit SBUF right behind the last matmul.
                    if final and ci == F // PS_CHUNK - 1:
                        h = PS_CHUNK // 2
                        nc.vector.tensor_copy(ot[:, pos:pos + h], acc[:, :h])
                        nc.scalar.copy(ot[:, pos + h:pos + PS_CHUNK],
                                       acc[:, h:])
                    else:
                        eng = copy_engines[ncopy % 2]
                        if eng is nc.scalar:
                            eng.copy(ot[:, pos:pos + PS_CHUNK], acc)
                        else:
                            eng.tensor_copy(ot[:, pos:pos + PS_CHUNK], acc)
                    ncopy += 1
                    # Store every 2048 cols as soon as its copies land: the
                    # DMA engines have ~50% idle mid-stream, so draining
                    # uniformly removes the multi-MB backlog that otherwise
                    # serializes after the final matmul. The last chunk goes
                    # as two 256KB pieces so the final drain is tiny.
                    if final and ci == F // PS_CHUNK - 1:
                        nc.sync.dma_start(out_d[c, :, pos - PS_CHUNK:pos],
                                          ot[:, pos - PS_CHUNK:pos])
                        h = PS_CHUNK // 2
                        nc.sync.dma_start(out_d[c, :, pos:pos + h],
                                          ot[:, pos:pos + h])
                        nc.sync.dma_start(out_d[c, :, pos + h:pos + PS_CHUNK],
                                          ot[:, pos + h:pos + PS_CHUNK])
                    elif ci % 2 == 1:
                        lo = pos - PS_CHUNK
                        hi = pos + PS_CHUNK
                        nc.sync.dma_start(out_d[c, :, lo:hi], ot[:, lo:hi])
    nc.compile()
    return nc


def kernel(values: np.ndarray, freq_hz: np.ndarray) -> np.ndarray:
    global LAST_RESULTS
    values = np.asarray(values, np.float32)
    freq_hz = np.asarray(freq_hz, np.float32)
    assert values.shape == (B, len(OP_CODES)) and freq_hz.shape == (F,)

    # Host precompute (tiny, fp64-exact): Laurent coefficients + omega powers
    coef = _host_coeffs(values)                              # (8, B, 13) f64
    om = 2.0 * np.pi * freq_hz.astype(np.float64)
    wt = om / OM0
    W = np.stack([wt ** (k - K0) for k in range(NK)]).astype(np.float32)
    W1 = W.astype(BF16)
    W2 = (W - W1.astype(np.float32)).astype(BF16)
    Wstack = np.concatenate([W1, W2, W1])                    # (39, F) bf16

    if "nc" not in _COMPILED:
        _COMPILED["nc"] = _build_module()
    nc = _COMPILED["nc"]

    in_maps = []
    for core in range(NCORES):
        sl = slice(core * BPC, (core + 1) * BPC)
        lhs = np.ascontiguousarray(
            np.transpose(coef[:, sl, :], (0, 2, 1))          # (8, 13, BPC)
        ).astype(np.float32)
        cstack = np.empty((KS, CB), BF16)
        for c in range(NCOMP):
            h = lhs[c].astype(BF16)
            lo = (lhs[c] - h.astype(np.float32)).astype(BF16)
            blk = cstack[:, c * BPC:(c + 1) * BPC]
            blk[0 * NK:1 * NK] = h
            blk[1 * NK:2 * NK] = h
            blk[2 * NK:3 * NK] = lo
        cwnp = np.empty((KS, CB + F), BF16)
        cwnp[:, :CB] = cstack
        cwnp[:, CB:] = Wstack
        in_maps.append({"cw": cwnp})

    res = run_bass_kernel_spmd(nc, in_maps, core_ids=list(range(NCORES)))
    LAST_RESULTS = res
    out = np.concatenate(
        [np.asarray(res.results[c]["out"]).astype(np.float32)
         for c in range(NCORES)], axis=1)
    return out



# revision 3
# speedup vs baseline: 1.0531x; 1.0531x over previous
"""Trainium2 kernel for nn_CascadedABCDCircuit: cascaded 2-port ABCD ladder.

Math: each stage multiplies the ABCD state by (I + s_i*G_i) with G_i nilpotent,
so every output component is a Laurent polynomial in omega (degree -6..+6, 13
coefficients) with batch-dependent coefficients. Host computes the coefficients
exactly in fp64 (tiny (1024,13) complex recurrence); the device evaluates
out[c,b,f] = sum_m C[c,b,m] * W[m,f] as matmuls and streams the result to HBM.

Precision: correctness gate is rel_err < 2e-2. Coefficients are split
hi+lo in bf16 (2-term, ~16-bit effective mantissa) and W is bf16; the
product accumulates in fp32 PSUM; output stores as bf16 (measured rel_l2
~2.3e-3 on the reference distribution). The 2-term split keeps the
contraction at K=26 <= 32, which is what unlocks the PE tiling below.

PE tiling: with K=26 the 128x128 array runs in 32x128 row-tiled mode: 4
independent matmuls (one per component) stream concurrently through the 4
row groups, quadrupling output rate vs a single K=39 matmul chain. The PE
clock is pinned at 1.2 GHz on this part (HAM never unthrottles; verified
over a 55us gapless stream), so this 4x in moving-column efficiency is the
only way to shrink PE time: stream drops ~54.6us -> ~14us of PE work.

Pipeline: per 512-col chunk-set, 4 components' matmuls fill the 4 bank
quarters of one [128,2048] PSUM tile (each quarter = exactly one bank, so
the concurrent row tiles never collide). Two such PSUM tiles ping-pong (all
8 banks). A single 2048-col PSUM->SBUF copy per set (f32->bf16 cast)
alternates DVE/ACT; sets are staged component-interleaved in SBUF and
stored to HBM in that interleaved layout (host de-interleaves for free).
The kernel is then store-bound: ~17MB of bf16 output per core at the
~310-358 GB/s per-core HBM limit. Stores are issued fine-grained early
(prime the SDMA queue the moment data exists) and coarser later.

Sharding: pure data-parallel over batch: 8 cores x 128 batches, every core
sees all 8192 freqs.
"""
import numpy as np
import sys

for _p in ("/opt/trn_rl_repo", "/root/.axon_site/_ro/trn_rl_repo"):
    if _p not in sys.path:
        sys.path.append(_p)

import ml_dtypes
import concourse.bacc as bacc
import concourse.mybir as mybir
from concourse import tile
from concourse.bass_utils import run_bass_kernel_spmd

# Problem constants (hardcoded per contract)
B, F = 1024, 8192
OP_CODES = [3, 0, 1, 2, 3, 0, 1, 2, 3, 0, 1, 2]
Q_L, Q_C = 50.0, 100.0
NK, K0 = 13, 6               # omega powers -6..+6
NCORES = 8
BPC = B // NCORES            # 128 batches per core
NCOMP = 8                    # Ar, Ai, Br, Bi, Cr, Ci, Dr, Di
OM0 = 2.0 * np.pi * np.sqrt(1e9 * 10e9)   # omega normalizer (geometric mid)

KS = 2 * NK                  # hi+lo stacked contraction dim (26)
MM_N = 512                   # moving cols per matmul (1 PSUM bank, fp32)
NSET = F // MM_N             # 16 chunk-sets per pass
NPASS = 2                    # components 0-3, then 4-7
SET_COLS = 4 * MM_N          # 2048 staged cols per set (4 comps x 512)
BF16 = ml_dtypes.bfloat16

LAST_RESULTS = None          # BassKernelResults of the most recent run
_COMPILED = {}


def _host_coeffs(values):
    """values (B,12) fp32 -> (NCOMP, B, NK) fp64 coeffs in powers of (om/OM0)."""
    v = values.astype(np.float64)
    nb = v.shape[0]
    A = np.zeros((nb, NK), np.complex128); A[:, K0] = 1.0
    Bm = np.zeros((nb, NK), np.complex128)
    Cm = np.zeros((nb, NK), np.complex128)
    Dm = np.zeros((nb, NK), np.complex128); Dm[:, K0] = 1.0

    def shift_mul(P, fac, dk):
        out = np.zeros_like(P)
        if dk == 1:
            out[:, 1:] = P[:, :-1]
        else:
            out[:, :-1] = P[:, 1:]
        return out * fac[:, None]

    for i, code in enumerate(OP_CODES):
        vi = v[:, i]
        if code == 0:      # series L
            fac = vi * OM0 * (1.0 / Q_L + 1j)
            Bm = Bm + shift_mul(A, fac, +1)
            Dm = Dm + shift_mul(Cm, fac, +1)
        elif code == 1:    # series C (reciprocal)
            c = (1.0 / Q_C - 1j) / (1.0 + 1.0 / Q_C**2)
            fac = c / (vi * OM0)
            Bm = Bm + shift_mul(A, fac, -1)
            Dm = Dm + shift_mul(Cm, fac, -1)
        elif code == 2:    # shunt L (reciprocal)
            c = (1.0 / Q_L - 1j) / (1.0 + 1.0 / Q_L**2)
            fac = c / (vi * OM0)
            A = A + shift_mul(Bm, fac, -1)
            Cm = Cm + shift_mul(Dm, fac, -1)
        else:              # shunt C
            fac = vi * OM0 * (1.0 / Q_C + 1j)
            A = A + shift_mul(Bm, fac, +1)
            Cm = Cm + shift_mul(Dm, fac, +1)
    return np.stack([A.real, A.imag, Bm.real, Bm.imag,
                     Cm.real, Cm.imag, Dm.real, Dm.imag])


# Store split schedule, in chunk-set index ranges per pass. Fine-grained at
# the start of pass 0 (prime the store pipe early) and at the very end
# (short final receipt), 2-set (1MB) pieces in the middle.
STORE_SPLITS = {
    0: [(0, 1), (1, 2), (2, 3), (3, 4), (4, 6), (6, 8),
        (8, 10), (10, 12), (12, 14), (14, 16)],
    1: [(0, 2), (2, 4), (4, 6), (6, 8), (8, 10), (10, 12),
        (12, 14), (14, 15), (15, 16)],
}


def _build_module():
    """SPMD module.

    Inputs:  cd [128, NPASS*BPC] bf16 — coefficients; partitions 32r+k hold
             hi (k<13) / lo (13<=k<26) of component 4p+r, zeros elsewhere.
             wd [128, F] bf16 — omega powers W1[k%13] replicated into each
             32-partition row group, zeros at k>=26.
    Output:  out [128, NPASS*NSET*SET_COLS] bf16, component-interleaved:
             col ((p*NSET+s)*4 + r)*512 + j  =  comp 4p+r, freq s*512+j.
    """
    nc = bacc.Bacc("TRN2", target_bir_lowering=False, debug=False,
                   enable_asserts=False, num_devices=NCORES)
    cd = nc.dram_tensor("cd", [128, NPASS * BPC], mybir.dt.bfloat16,
                        kind="ExternalInput")
    wd = nc.dram_tensor("wd", [128, F], mybir.dt.bfloat16,
                        kind="ExternalInput")
    out_d = nc.dram_tensor("out", [128, NPASS * F * 4], mybir.dt.bfloat16,
                           kind="ExternalOutput")

    with tile.TileContext(nc) as tc:
        with (
            tc.tile_pool(name="const", bufs=1) as cpool,
            tc.tile_pool(name="ps", bufs=2, space="PSUM") as pspool,
        ):
            ct = cpool.tile([128, NPASS * BPC], mybir.dt.bfloat16)
            # W chunk tiles sized so the first matmul's operands are tiny
            # and each tile's consumers only wait on their own DMA.
            w_chunks = [(0, 512), (512, 2048), (2048, 4096),
                        (4096, 6144), (6144, 8192)]
            wt = [cpool.tile([128, hi - lo], mybir.dt.bfloat16,
                             name=f"wt{ti}")
                  for ti, (lo, hi) in enumerate(w_chunks)]
            ot = cpool.tile([128, NPASS * F * 4], mybir.dt.bfloat16)

            # Input loads: coeffs + first W chunk land first (first matmul's
            # operands), the rest stream in ahead of use on both HWDGE rings.
            nc.scalar.dma_start(ct[:, :], cd[:, :])
            nc.sync.dma_start(wt[0][:, :], wd[:, 0:512])
            nc.sync.dma_start(wt[1][:, :], wd[:, 512:2048])
            nc.scalar.dma_start(wt[2][:, :], wd[:, 2048:4096])
            nc.scalar.dma_start(wt[3][:, :], wd[:, 4096:6144])
            nc.scalar.dma_start(wt[4][:, :], wd[:, 6144:8192])

            def w_slice(s):
                col = s * MM_N
                for ti, (lo, hi) in enumerate(w_chunks):
                    if lo <= col < hi:
                        return wt[ti], col - lo
                raise AssertionError(col)

            nset_done = 0
            for p in range(NPASS):
                pend = [(a, b) for a, b in STORE_SPLITS[p]]
                for s in range(NSET):
                    pt = pspool.tile([128, SET_COLS], mybir.dt.float32)
                    wtile, off = w_slice(s)
                    for r in range(4):
                        # comp 4p+r on row group r -> bank quarter r
                        nc.tensor.matmul(
                            pt[:, r * MM_N:(r + 1) * MM_N],
                            ct[32 * r:32 * r + KS,
                               p * BPC:(p + 1) * BPC],
                            wtile[32 * r:32 * r + KS, off:off + MM_N],
                            tile_position=(32 * r, 0),
                        )
                    dst_lo = (p * NSET + s) * SET_COLS
                    dst = ot[:, dst_lo:dst_lo + SET_COLS]
                    # single 2048-col PSUM->SBUF drain with f32->bf16 cast;
                    # DVE/ACT are the only engines with a PSUM port.
                    if nset_done % 2 == 0:
                        nc.vector.tensor_copy(dst, pt)
                    else:
                        nc.scalar.copy(dst, pt)
                    nset_done += 1
                    # stores stream out as soon as their sets are staged
                    if pend and s == pend[0][1] - 1:
                        a, b = pend.pop(0)
                        lo = (p * NSET + a) * SET_COLS
                        hi = (p * NSET + b) * SET_COLS
                        nc.sync.dma_start(out_d[:, lo:hi], ot[:, lo:hi])
    nc.compile()
    return nc


def kernel(values: np.ndarray, freq_hz: np.ndarray) -> np.ndarray:
    global LAST_RESULTS
    values = np.asarray(values, np.float32)
    freq_hz = np.asarray(freq_hz, np.float32)
    assert values.shape == (B, len(OP_CODES)) and freq_hz.shape == (F,)

    # Host precompute (tiny, fp64-exact): Laurent coefficients + omega powers
    coef = _host_coeffs(values)                              # (8, B, 13) f64
    om = 2.0 * np.pi * freq_hz.astype(np.float64)
    wt = om / OM0
    W = np.stack([wt ** (k - K0) for k in range(NK)])        # (13, F) f64
    W1 = W.astype(np.float32).astype(BF16)
    wd = np.zeros((128, F), BF16)
    for r in range(4):
        wd[32 * r:32 * r + NK] = W1
        wd[32 * r + NK:32 * r + KS] = W1
    wd = np.ascontiguousarray(wd)

    if "nc" not in _COMPILED:
        _COMPILED["nc"] = _build_module()
    nc = _COMPILED["nc"]

    in_maps = []
    for core in range(NCORES):
        sl = slice(core * BPC, (core + 1) * BPC)
        lhs = np.ascontiguousarray(
            np.transpose(coef[:, sl, :], (0, 2, 1))          # (8, 13, BPC)
        ).astype(np.float32)
        h = lhs.astype(BF16)
        lo = (lhs - h.astype(np.float32)).astype(BF16)
        cd = np.zeros((128, NPASS * BPC), BF16)
        for p in range(NPASS):
            for r in range(4):
                c = 4 * p + r
                cd[32 * r:32 * r + NK, p * BPC:(p + 1) * BPC] = h[c]
                cd[32 * r + NK:32 * r + KS, p * BPC:(p + 1) * BPC] = lo[c]
        in_maps.append({"cd": cd, "wd": wd})

    res = run_bass_kernel_spmd(nc, in_maps, core_ids=list(range(NCORES)))
    LAST_RESULTS = res
    parts = []
    for core in range(NCORES):
        dev = np.asarray(res.results[core]["out"])           # (128, 65536) bf16
        arr = dev.reshape(BPC, NPASS, NSET, 4, MM_N)
        # [b, p, s, r, j] -> [p, r, b, s, j] -> (8, BPC, F)
        parts.append(arr.transpose(1, 3, 0, 2, 4)
                     .reshape(NCOMP, BPC, F).astype(np.float32))
    return np.concatenate(parts, axis=1)
